# revision 13
# baseline (speedup 1.0000x reference)
"""ChildSum TreeGRU on 8 Trainium2 NeuronCores.

Data-parallel over trees (16 trees/core). On-device layout is feature-major
([256 feat] -> 2x128 partitions, nodes on the free dim); the host transposes
x's leaf slice in and the output back out. All matmuls run as float32r.

h state lives in merged-halves tiles [128, 2, cols] (half-major free dim) so
every elementwise op covers both feature halves in one DVE instruction.

Heap tree, per-core column order is tree-major: col = tree*len + in-level pos.
Levels 10(leaves)..6 are processed per group of 4 trees; level-6 results land
in a joint buffer covering heap nodes 0..126 (levels 0..6) for all 16 trees;
levels 5..0 are then processed jointly, each level streamed out as computed.
"""
import sys

for p in ("/opt/trn_rl_repo", "/root/.axon_site/_ro/trn_rl_repo"):
    if p not in sys.path:
        sys.path.insert(0, p)

import numpy as np
import concourse.tile as tile
from concourse import bacc, mybir
from concourse.bass_utils import run_bass_kernel_spmd

f32 = mybir.dt.float32
f32r = mybir.dt.float32r
AF = mybir.ActivationFunctionType
ALU = mybir.AluOpType

T, DEPTH, NN, H = 128, 11, 2047, 256
NCORES = 8
TPC = T // NCORES          # 16 trees per core
G = 4                      # trees per group
NG = TPC // G              # 4 groups
NLEAF = 1 << (DEPTH - 1)   # 1024
LEAF0 = NLEAF - 1          # 1023
JN = (1 << 7) - 1          # 127 nodes/tree in the joint buffer (levels 0..6)
PS_COLS = 1024             # psum batch (2 banks) consumed by one ACT
WIN = 1024                 # parent-column window for level pipelining


def _emit_level(nc, P, tag, NT, Lct, hview, hnode, out_slice, Wt, bias):
    """One GRU level for NT trees with Lct children per tree.

    hview(k, a, b): per-half 2D AP [128, b-a] over flat child cols (matmul rhs)
    hnode(a, b) -> (evens, odds, full) merged views over flat child cols
    out_slice(w0, w1): merged f32r output AP [128, 2, w1-w0] for parent cols
    """
    Lc = NT * Lct
    Lp = Lc // 2
    uzT, urT, ucT = Wt["uz"], Wt["ur"], Wt["uc"]
    bz, br, bc = bias["bz"], bias["br"], bias["bc"]

    def mm_into(ps, off, lhs, rhs_pair):
        n = rhs_pair[0].free_size()
        nc.tensor.matmul(ps[:, off:off + n], lhs[0], rhs_pair[0], start=True, stop=False)
        nc.tensor.matmul(ps[:, off:off + n], lhs[1], rhs_pair[1], start=False, stop=True)

    for w0 in range(0, Lp, WIN):
        wn = min(WIN, Lp - w0)
        wt_ = f"{tag}w{w0}"
        c0, cn = 2 * w0, 2 * wn

        # h_sum = hc_even + hc_odd  (merged halves, f32r)
        hs = P["hs"].tile([128, 2, wn], f32r, name=f"hs{wt_}", tag="hs")
        ev, od, full = hnode(c0, c0 + cn)
        nc.vector.tensor_tensor(hs[:], ev, od, ALU.add)

        # r = sigmoid(Ur @ h_sum + br)
        r = P["r"].tile([128, 2, wn], f32, name=f"r{wt_}", tag="r")
        for m in range(2):
            lhs = [urT[k][:, m * 128:(m + 1) * 128] for k in range(2)]
            ps = P["psrc"].tile([128, wn], f32, name=f"psr{wt_}_{m}", tag="psrc")
            for q0 in range(0, wn, 512):
                qn = min(512, wn - q0)
                mm_into(ps, q0, lhs, [hs[:, k, q0:q0 + qn] for k in range(2)])
            nc.scalar.activation(r[:, m, :], ps[:], AF.Sigmoid, bias=br[m][:])

        # z = sigmoid(Uz @ hc + bz) over this window's children
        z = P["z"].tile([128, 2, cn], f32, name=f"z{wt_}", tag="z")
        for m in range(2):
            lhs = [uzT[k][:, m * 128:(m + 1) * 128] for k in range(2)]
            for p0 in range(0, cn, PS_COLS):
                pn = min(PS_COLS, cn - p0)
                ps = P["psz"].tile([128, pn], f32, name=f"psz{wt_}_{m}_{p0}", tag="psz")
                for q0 in range(p0, p0 + pn, 512):
                    qn = min(512, p0 + pn - q0)
                    mm_into(ps, q0 - p0, lhs,
                            [hview(k, c0 + q0, c0 + q0 + qn) for k in range(2)])
                nc.scalar.activation(z[:, m, p0:p0 + pn], ps[:], AF.Sigmoid, bias=bz[m][:])

        # rh = r * h_sum (in place into hs, stays f32r)
        nc.vector.tensor_tensor(hs[:], r[:], hs[:], ALU.mult)

        # h_cand = tanh(Uc @ rh + bc)
        hcand = P["hc"].tile([128, 2, wn], f32, name=f"hcand{wt_}", tag="hc")
        for m in range(2):
            lhs = [ucT[k][:, m * 128:(m + 1) * 128] for k in range(2)]
            ps = P["psrc"].tile([128, wn], f32, name=f"psc{wt_}_{m}", tag="psrc")
            for q0 in range(0, wn, 512):
                qn = min(512, wn - q0)
                mm_into(ps, q0, lhs, [hs[:, k, q0:q0 + qn] for k in range(2)])
            nc.scalar.activation(hcand[:, m, :], ps[:], AF.Tanh, bias=bc[m][:])

        # gate combine
        z3 = z[:].rearrange("p h (x two) -> p h x two", two=2)
        zs = P["r"].tile([128, 2, wn], f32, name=f"zs{wt_}", tag="r")
        nc.vector.tensor_tensor(zs[:], z3[:, :, :, 0], z3[:, :, :, 1], ALU.add)
        # zh = z * hc in place into z
        nc.vector.tensor_tensor(z[:], z[:], full, ALU.mult)
        zhs = P["hs"].tile([128, 2, wn], f32, name=f"zhs{wt_}", tag="hs")
        nc.vector.tensor_tensor(zhs[:], z3[:, :, :, 0], z3[:, :, :, 1], ALU.add)
        # t = (zs - 1) * h_cand, in place into hcand
        nc.vector.scalar_tensor_tensor(hcand[:], zs[:], 1.0, hcand[:], ALU.subtract, ALU.mult)
        # h_new = zh_sum - t
        nc.vector.tensor_tensor(out_slice(w0, w0 + wn), zhs[:], hcand[:], ALU.subtract)


def _mk_group_views(htile):
    """Accessors for a merged contiguous level tile [128, 2, cols]."""
    def hview(k, a, b):
        return htile[:, k, a:b]

    def hnode(a, b):
        v = htile[:, :, a:b].rearrange("p h (x two) -> p h x two", two=2)
        return v[:, :, :, 0], v[:, :, :, 1], htile[:, :, a:b]
    return hview, hnode


def _build():
    nc = bacc.Bacc("TRN2", debug=False)

    xT_d = nc.dram_tensor("xT", [H, TPC * NLEAF], f32r, kind="ExternalInput")
    wT_d = nc.dram_tensor("wT", [H, H], f32r, kind="ExternalInput")
    uzT_d = nc.dram_tensor("uzT", [H, H], f32r, kind="ExternalInput")
    urT_d = nc.dram_tensor("urT", [H, H], f32r, kind="ExternalInput")
    ucT_d = nc.dram_tensor("ucT", [H, H], f32r, kind="ExternalInput")
    bw_d = nc.dram_tensor("bw", [H, 1], f32, kind="ExternalInput")
    bz_d = nc.dram_tensor("bz", [H, 1], f32, kind="ExternalInput")
    br_d = nc.dram_tensor("br", [H, 1], f32, kind="ExternalInput")
    bc_d = nc.dram_tensor("bc", [H, 1], f32, kind="ExternalInput")
    hout_d = nc.dram_tensor("h_out", [H, TPC, NN], f32, kind="ExternalOutput")

    with tile.TileContext(nc) as tc:
        from contextlib import ExitStack
        with ExitStack() as ctx:
            P = {}
            P["const"] = ctx.enter_context(tc.tile_pool(name="const", bufs=1))
            P["xg"] = ctx.enter_context(tc.tile_pool(name="xg", bufs=2))
            P["h10"] = ctx.enter_context(tc.tile_pool(name="h10", bufs=1))
            P["hl"] = ctx.enter_context(tc.tile_pool(name="hl", bufs=1))
            P["jbuf"] = ctx.enter_context(tc.tile_pool(name="jbuf", bufs=1))
            P["z"] = ctx.enter_context(tc.tile_pool(name="z", bufs=2))
            P["hs"] = ctx.enter_context(tc.tile_pool(name="hs", bufs=2))
            P["r"] = ctx.enter_context(tc.tile_pool(name="r", bufs=2))
            P["hc"] = ctx.enter_context(tc.tile_pool(name="hc", bufs=2))
            P["psz"] = ctx.enter_context(tc.tile_pool(name="psz", bufs=2, space="PSUM"))
            P["psrc"] = ctx.enter_context(tc.tile_pool(name="psrc", bufs=2, space="PSUM"))

            cp = P["const"]
            Wt = {}
            for nm, d in (("w", wT_d), ("uz", uzT_d), ("ur", urT_d), ("uc", ucT_d)):
                Wt[nm] = [cp.tile([128, H], f32r, name=f"{nm}T{k}") for k in range(2)]
                for k in range(2):
                    nc.sync.dma_start(Wt[nm][k][:], d.ap()[k * 128:(k + 1) * 128, :])
            bias = {}
            for nm, d in (("bw", bw_d), ("bz", bz_d), ("br", br_d), ("bc", bc_d)):
                bias[nm] = [cp.tile([128, 1], f32, name=f"{nm}{m}") for m in range(2)]
                for m in range(2):
                    nc.sync.dma_start(bias[nm][m][:], d.ap()[m * 128:(m + 1) * 128, :])

            # joint buffer: heap nodes 0..126 for all 16 trees, merged halves
            jbuf = P["jbuf"].tile([128, 2, TPC * JN], f32r, name="jbuf")
            jv = jbuf[:].rearrange("p h (t n) -> p h t n", t=TPC)

            for g in range(NG):
                gt = f"g{g}"
                # ---- leaf phase: h10 = tanh(W @ x + bw) ----
                xg = [P["xg"].tile([128, G * NLEAF], f32r, name=f"x{gt}_{k}", tag="xg")
                      for k in range(2)]
                for k in range(2):
                    for piece in range(0, G * NLEAF, 1024):
                        pend = min(piece + 1024, G * NLEAF)
                        nc.sync.dma_start(
                            xg[k][:, piece:pend],
                            xT_d.ap()[k * 128:(k + 1) * 128,
                                      g * G * NLEAF + piece:g * G * NLEAF + pend])
                h10 = P["h10"].tile([128, 2, G * NLEAF], f32r, name=f"h10{gt}", tag="h10")
                for m in range(2):
                    lhs = [Wt["w"][k][:, m * 128:(m + 1) * 128] for k in range(2)]
                    for p0 in range(0, G * NLEAF, PS_COLS):
                        pn = min(PS_COLS, G * NLEAF - p0)
                        ps = P["psz"].tile([128, pn], f32, name=f"psx{gt}_{m}_{p0}", tag="psz")
                        for q0 in range(p0, p0 + pn, 512):
                            qn = min(512, p0 + pn - q0)
                            nc.tensor.matmul(ps[:, q0 - p0:q0 - p0 + qn], lhs[0],
                                             xg[0][:, q0:q0 + qn], start=True, stop=False)
                            nc.tensor.matmul(ps[:, q0 - p0:q0 - p0 + qn], lhs[1],
                                             xg[1][:, q0:q0 + qn], start=False, stop=True)
                        nc.scalar.activation(h10[:, m, p0:p0 + pn], ps[:], AF.Tanh,
                                             bias=bias["bw"][m][:])
                for m in range(2):
                    nc.sync.dma_start(
                        hout_d.ap()[m * 128:(m + 1) * 128, g * G:(g + 1) * G,
                                    LEAF0:LEAF0 + NLEAF],
                        h10[:, m, :].rearrange("p (t n) -> p t n", t=G).bitcast(f32))

                # ---- levels 9..6 for this group ----
                hchild = h10
                for lv in range(DEPTH - 2, 5, -1):
                    Lct = 2 ** (lv + 1)
                    Lpt = 2 ** lv
                    hview, hnode = _mk_group_views(hchild)
                    if lv == 6:
                        def out_slice(a, b, _g=g, _Lpt=Lpt):
                            assert a == 0 and b == G * _Lpt
                            return jv[:, :, _g * G:(_g + 1) * G, _Lpt - 1:2 * _Lpt - 1]
                        hnew = None
                    else:
                        hnew = P["hl"].tile([128, 2, G * Lpt], f32r,
                                            name=f"h{lv}{gt}", tag=f"h{lv}")
                        def out_slice(a, b, _t=hnew):
                            return _t[:, :, a:b]
                    _emit_level(nc, P, f"{gt}l{lv}", G, Lct, hview, hnode,
                                out_slice, Wt, bias)
                    if lv > 6:
                        for m in range(2):
                            nc.sync.dma_start(
                                hout_d.ap()[m * 128:(m + 1) * 128, g * G:(g + 1) * G,
                                            Lpt - 1:2 * Lpt - 1],
                                hnew[:, m, :].rearrange("p (t n) -> p t n", t=G).bitcast(f32))
                        hchild = hnew

            # l6 region of the joint buffer is complete: stream it out
            for m in range(2):
                nc.sync.dma_start(
                    hout_d.ap()[m * 128:(m + 1) * 128, :, 63:JN],
                    jv[:, m, :, 63:JN].bitcast(f32))

            # ---- joint levels 5..0 over jbuf, streaming each level out ----
            for lv in range(5, -1, -1):
                Lct = 2 ** (lv + 1)
                Lpt = 2 ** lv

                def hview(k, a, b, _Lct=Lct):
                    # chunks are tree-aligned for the joint levels
                    assert a % _Lct == 0 and b % _Lct == 0
                    return jv[:, k, a // _Lct:b // _Lct, _Lct - 1:2 * _Lct - 1]

                def hnode(a, b, _Lct=Lct):
                    assert a == 0 and b == TPC * _Lct
                    v = jv[:, :, :, _Lct - 1:2 * _Lct - 1]
                    ev = v.rearrange("p h t (x two) -> p h t x two", two=2)
                    return ev[:, :, :, :, 0], ev[:, :, :, :, 1], v

                def out_slice(a, b, _Lpt=Lpt):
                    assert a == 0 and b == TPC * _Lpt
                    return jv[:, :, :, _Lpt - 1:2 * _Lpt - 1]

                _emit_level(nc, P, f"j{lv}", TPC, Lct, hview, hnode, out_slice, Wt, bias)
                for m in range(2):
                    nc.sync.dma_start(
                        hout_d.ap()[m * 128:(m + 1) * 128, :, Lpt - 1:2 * Lpt - 1],
                        jv[:, m, :, Lpt - 1:2 * Lpt - 1].bitcast(f32))

    nc.compile()
    return nc


_NC = None


def _get_nc():
    global _NC
    if _NC is None:
        _NC = _build()
    return _NC


def make_in_maps(inputs):
    x = np.asarray(inputs["x"], np.float32)
    W = np.asarray(inputs["W"], np.float32)
    bW = np.asarray(inputs["bW"], np.float32).reshape(H, 1)
    Ur = np.asarray(inputs["Ur"], np.float32)
    br = np.asarray(inputs["br"], np.float32).reshape(H, 1)
    Uc = np.asarray(inputs["Uc"], np.float32)
    bc = np.asarray(inputs["bc"], np.float32).reshape(H, 1)
    Uz = np.asarray(inputs["Uz"], np.float32)
    bz = np.asarray(inputs["bz"], np.float32).reshape(H, 1)
    shared = {
        "wT": np.ascontiguousarray(W.T), "uzT": np.ascontiguousarray(Uz.T),
        "urT": np.ascontiguousarray(Ur.T), "ucT": np.ascontiguousarray(Uc.T),
        "bw": bW, "bz": bz, "br": br, "bc": bc,
    }
    in_maps = []
    for c in range(NCORES):
        xs = x[c * TPC:(c + 1) * TPC, LEAF0:, :]          # [16, 1024, 256]
        xTc = np.ascontiguousarray(xs.transpose(2, 0, 1)).reshape(H, TPC * NLEAF)
        in_maps.append({"xT": xTc, **shared})
    return in_maps


def assemble_out(core_outs):
    out = np.empty((T, NN, H), np.float32)
    for c in range(NCORES):
        # [256, 16, 2047] -> [16, 2047, 256]
        out[c * TPC:(c + 1) * TPC] = core_outs[c].transpose(1, 2, 0)
    return out


def kernel(**inputs):
    nc = _get_nc()
    in_maps = make_in_maps(inputs)
    res = run_bass_kernel_spmd(nc, in_maps, list(range(NCORES)))
    return assemble_out([r["h_out"] for r in res.results])
